# revision 12
# baseline (speedup 1.0000x reference)
"""Multi-head causal attention (B=4, T=2048, C=1024, H=16, DH=64) on 8 trn2
NeuronCores.

Sharding: core = (batch, head-half): core 2*b+g computes heads g*8..g*8+8 of
batch b, including the partial output projection with the matching 512 rows
of Wp (tensor-parallel). Host-side unshard sums the two partials per batch
and adds bp.

Per-core kernel (all matmuls fp32r = full PE speed, ~1e-4 accuracy):
  phase 1: QKV projections from x^T (c on partitions).
           Q^T/K^T produced in [d, t] layout (heads pair-packed, 128 rows),
           V in [s, d] layout with a ones-column per head (softmax sums).
           Q^T spills to DRAM scratch (SBUF pressure), K^T and V resident.
  phase 2: per head-pair, per 512-wide query chunk tj, stream key chunks
           si (128 wide): S^T = K Q^T via row-tiled matmul pair (K=64 each),
           causal tril mask add on diagonal chunks, exp on ScalarE
           (scale=1/32 fused) -> P^T fp32r, AV accumulates
           O^T[65, 512] = [V | 1]^T P^T in PSUM (row 64 = L sums).
           Normalize via DRAM-bounce broadcast of L + reciprocal + multiply.
  phase 3: output projection back to natural [t, e] layout, DMA out.
"""
import numpy as np

import concourse.bass as bass
import concourse.mybir as mybir
import concourse.tile as tile
from concourse import bacc, bass_utils

F32 = mybir.dt.float32
F32R = mybir.dt.float32r

B, T, C, H, DH = 4, 2048, 1024, 16, 64
HG = H // 2          # heads per core (8)
CC = C // 128        # contraction chunks (8)
TJ = 512             # query chunk width
NTJ = T // TJ        # 4
NSI = T // 128       # 16 key chunks
MASK_VAL = -1.0e6
SCALE = 1.0 / 32.0   # 1/sqrt(C)

TRACE = False
_NC_CACHE = {}


def _build():
    nc = bacc.Bacc(trn_type="TRN2", target_bir_lowering=False, debug=False)

    xT = nc.dram_tensor("xT", [C, T], F32R, kind="ExternalInput")
    wqkv = nc.dram_tensor("wqkv", [C, 3 * HG * DH], F32R, kind="ExternalInput")
    wp = nc.dram_tensor("wp", [HG * DH, C], F32R, kind="ExternalInput")
    tril = nc.dram_tensor("tril", [128, 128], F32, kind="ExternalInput")
    ones8 = nc.dram_tensor("ones8", [128, HG], F32R, kind="ExternalInput")
    out = nc.dram_tensor("out", [T, C], F32, kind="ExternalOutput")

    q_dram = nc.dram_tensor("q_scratch", [HG * DH, T], F32R)
    l_dram = nc.dram_tensor("l_scratch", [HG * NTJ, TJ], F32)

    with tile.TileContext(nc) as tc:
        with (
            tc.tile_pool(name="persist", bufs=1) as persist,
            tc.tile_pool(name="qstage", bufs=3) as qstage,
        ):
            tril_sb = persist.tile([128, 128], F32)
            nc.sync.dma_start(out=tril_sb, in_=tril.ap())
            ones_sb = persist.tile([128, HG], F32R)
            nc.sync.dma_start(out=ones_sb, in_=ones8.ap())

            k_sb = [persist.tile([128, T], F32R, name=f"k_{mg}")
                    for mg in range(HG // 2)]
            v_sb = [persist.tile([128, HG, DH + 1], F32R, name=f"v_{si}")
                    for si in range(NSI)]

            # ---- phase 1: projections ----
            with (
                tc.tile_pool(name="xw", bufs=1) as xw,
                tc.tile_pool(name="pps", bufs=4, space="PSUM") as pps,
            ):
                w_sb = [xw.tile([128, 3 * HG * DH], F32R, name=f"w_{c}")
                        for c in range(CC)]
                xt_sb = [xw.tile([128, T], F32R, name=f"xt_{c}")
                         for c in range(CC)]

                # DMA in consumption order: q-weights, xt t-chunks, k/v weights
                for c in range(CC):
                    csl = slice(c * 128, (c + 1) * 128)
                    nc.sync.dma_start(out=w_sb[c][:, 0:512],
                                      in_=wqkv.ap()[csl, 0:512])
                    nc.sync.dma_start(out=xt_sb[c][:, 0:TJ],
                                      in_=xT.ap()[csl, 0:TJ])
                for tn in range(1, NTJ):
                    tsl = slice(tn * TJ, (tn + 1) * TJ)
                    for c in range(CC):
                        csl = slice(c * 128, (c + 1) * 128)
                        nc.sync.dma_start(out=xt_sb[c][:, tsl],
                                          in_=xT.ap()[csl, tsl])
                for c in range(CC):
                    csl = slice(c * 128, (c + 1) * 128)
                    nc.sync.dma_start(out=w_sb[c][:, 512:1024],
                                      in_=wqkv.ap()[csl, 512:1024])
                for c in range(CC):
                    csl = slice(c * 128, (c + 1) * 128)
                    nc.sync.dma_start(out=w_sb[c][:, 1024:1536],
                                      in_=wqkv.ap()[csl, 1024:1536])

                # Q: out [d(128 = head pair), t(512)], lhsT = W cols.
                for tn in range(NTJ):
                    tsl = slice(tn * TJ, (tn + 1) * TJ)
                    for mg in range(HG // 2):
                        qp = pps.tile([128, TJ], F32, name="qp", tag="pp")
                        for c in range(CC):
                            nc.tensor.matmul(
                                qp, w_sb[c][:, mg * 128:(mg + 1) * 128],
                                xt_sb[c][:, tsl],
                                start=(c == 0), stop=(c == CC - 1))
                        qs = qstage.tile([128, TJ], F32R, name="qs")
                        nc.scalar.copy(qs, qp)
                        nc.sync.dma_start(
                            out=q_dram.ap()[mg * 128:(mg + 1) * 128, tsl], in_=qs)
                # K
                for tn in range(NTJ):
                    tsl = slice(tn * TJ, (tn + 1) * TJ)
                    for mg in range(HG // 2):
                        kp = pps.tile([128, TJ], F32, name="kp", tag="pp")
                        for c in range(CC):
                            nc.tensor.matmul(
                                kp, w_sb[c][:, 512 + mg * 128:512 + (mg + 1) * 128],
                                xt_sb[c][:, tsl],
                                start=(c == 0), stop=(c == CC - 1))
                        nc.vector.tensor_copy(k_sb[mg][:, tsl], kp)

                # V: out [s(128), d(512 = 8 heads x 64)], lhsT = x^T cols.
                for si in range(NSI):
                    ssl = slice(si * 128, (si + 1) * 128)
                    vp = pps.tile([128, HG * DH], F32, name="vp", tag="pp")
                    for c in range(CC):
                        nc.tensor.matmul(
                            vp, xt_sb[c][:, ssl], w_sb[c][:, 1024:1536],
                            start=(c == 0), stop=(c == CC - 1))
                    nc.vector.tensor_copy(
                        v_sb[si][:, :, 0:DH],
                        vp.rearrange("p (h d) -> p h d", h=HG))
                    nc.vector.tensor_copy(
                        out=v_sb[si][:, :, DH:DH + 1], in_=ones_sb[:, :, None])

            # ---- phases 2+3 pools (reuse the released xw zone) ----
            with (
                tc.tile_pool(name="late", bufs=1) as late,
                tc.tile_pool(name="ppool", bufs=3) as ppool,
                tc.tile_pool(name="npool", bufs=3) as npool,
                tc.tile_pool(name="outpool", bufs=3) as outpool,
                tc.tile_pool(name="aps", bufs=2, space="PSUM") as aps,
            ):
                wp_sb = []
                for hp in range(HG // 2):
                    t_ = late.tile([128, C], F32R, name=f"wp_{hp}")
                    nc.sync.dma_start(out=t_, in_=wp.ap()[hp * 128:(hp + 1) * 128, :])
                    wp_sb.append(t_)
                o_sb = [late.tile([128, T], F32R, name=f"o_{hp}")
                        for hp in range(HG // 2)]

                # ---- phase 2: attention, two head-pair units interleaved ----
                def attn_unit_setup(hp, tj):
                    tsl = slice(tj * TJ, (tj + 1) * TJ)
                    qt = qstage.tile([128, TJ], F32R, name="qt")
                    nc.sync.dma_start(
                        out=qt, in_=q_dram.ap()[hp * 128:(hp + 1) * 128, tsl])
                    o_psA = aps.tile([DH + 1, TJ], F32, name="o_ps0", bufs=1)
                    o_psB = aps.tile([DH + 1, TJ], F32, name="o_ps1", bufs=1)
                    return (hp, qt, o_psA, o_psB)

                def attn_chunk(unit, tj, si, nsi):
                    hp, qt, o_ps0, o_ps1 = unit
                    h0, h1 = 2 * hp, 2 * hp + 1
                    r = si - 4 * tj
                    toff = 0 if r < 0 else 128 * r
                    ssl = slice(si * 128, (si + 1) * 128)

                    s_ps = aps.tile([128, 2, TJ], F32, name="s_ps", bufs=3)
                    nc.tensor.matmul(
                        s_ps[:, 0, toff:TJ],
                        k_sb[hp][0:64, ssl], qt[0:64, toff:TJ],
                        start=True, stop=True, tile_position=(0, 0))
                    nc.tensor.matmul(
                        s_ps[:, 1, toff:TJ],
                        k_sb[hp][64:128, ssl], qt[64:128, toff:TJ],
                        start=True, stop=True, tile_position=(64, 0))
                    if r >= 0:
                        nc.vector.tensor_tensor(
                            out=s_ps[:, :, toff:toff + 128],
                            in0=s_ps[:, :, toff:toff + 128],
                            in1=tril_sb[:, None, :].to_broadcast(
                                (128, 2, 128)),
                            op=mybir.AluOpType.add)
                    p_sb = ppool.tile([128, 2, TJ], F32R, name="p_sb")
                    nc.scalar.activation(
                        p_sb[:, :, toff:TJ], s_ps[:, :, toff:TJ],
                        mybir.ActivationFunctionType.Exp, scale=SCALE)
                    nc.tensor.matmul(
                        o_ps0[:, toff:TJ], v_sb[si][:, h0, :],
                        p_sb[:, 0, toff:TJ],
                        start=(si == 0), stop=(si == nsi - 1))
                    nc.tensor.matmul(
                        o_ps1[:, toff:TJ], v_sb[si][:, h1, :],
                        p_sb[:, 1, toff:TJ],
                        start=(si == 0), stop=(si == nsi - 1))

                def attn_norm(unit, tj):
                    # normalize: divide rows 0..63 by row 64 (L sums).
                    # Copy PSUM->SBUF first so the o_ps slot frees without
                    # waiting for the L DMA-broadcast roundtrip.
                    hp, qt, o_ps0, o_ps1 = unit
                    tsl = slice(tj * TJ, (tj + 1) * TJ)
                    for idx, o_ps in ((0, o_ps0), (1, o_ps1)):
                        lrow = (hp * 2 + idx) * NTJ + tj
                        o_stage = npool.tile([DH + 1, TJ], F32,
                                             name="o_stage")
                        nc.vector.tensor_copy(o_stage, o_ps)
                        nc.sync.dma_start(
                            out=l_dram.ap()[lrow:lrow + 1, :],
                            in_=o_stage[DH:DH + 1, :])
                        lb = npool.tile([64, TJ], F32, name="lb")
                        nc.sync.dma_start(
                            out=lb,
                            in_=l_dram.ap()[lrow:lrow + 1, :]
                            .to_broadcast((64, TJ)))
                        linv = npool.tile([64, TJ], F32, name="linv")
                        nc.vector.reciprocal_approx_fast(linv, lb)
                        if idx == 0:
                            nc.vector.tensor_tensor(
                                out=o_sb[hp][0:64, tsl],
                                in0=o_stage[0:DH, :],
                                in1=linv, op=mybir.AluOpType.mult)
                        else:
                            o_tmp = npool.tile([64, TJ], F32R, name="o_tmp")
                            nc.vector.tensor_tensor(
                                out=o_tmp, in0=o_stage[0:DH, :],
                                in1=linv, op=mybir.AluOpType.mult)
                            nc.sync.dma_start(
                                out=o_sb[hp][64:128, tsl], in_=o_tmp)

                def proj_tile(ti, en):
                    tsl = slice(ti * 128, (ti + 1) * 128)
                    esl = slice(en * TJ, (en + 1) * TJ)
                    op_ps = aps.tile([128, TJ], F32, name="op_ps",
                                     tag="s_ps", bufs=3)
                    for hp in range(HG // 2):
                        nc.tensor.matmul(
                            op_ps, o_sb[hp][:, tsl], wp_sb[hp][:, esl],
                            start=(hp == 0), stop=(hp == HG // 2 - 1))
                    ob = outpool.tile([128, TJ], F32, name="ob")
                    nc.scalar.copy(ob, op_ps)
                    nc.sync.dma_start(out=out.ap()[tsl, esl], in_=ob)

                for hp in range(HG // 2):
                    for tj in range(NTJ):
                        unit = attn_unit_setup(hp, tj)
                        nsi = 4 * tj + 4
                        for si in range(nsi):
                            attn_chunk(unit, tj, si, nsi)
                        attn_norm(unit, tj)
                for ti in range(T // 128):
                    for en in range(C // TJ):
                        proj_tile(ti, en)

    nc.compile()
    return nc


def _get_nc():
    if "nc" not in _NC_CACHE:
        _NC_CACHE["nc"] = _build()
    return _NC_CACHE["nc"]


def _make_in_maps(x, Wq, Wk, Wv, Wp):
    tril_h = np.where(
        np.arange(128)[:, None] > np.arange(128)[None, :],
        np.float32(MASK_VAL), np.float32(0.0)).astype(np.float32)
    in_maps = []
    for core in range(8):
        b, g = core // 2, core % 2
        heads = range(g * HG, (g + 1) * HG)
        wq = np.concatenate([Wq[h] for h in heads], axis=1)
        wk = np.concatenate([Wk[h] for h in heads], axis=1)
        wv = np.concatenate([Wv[h] for h in heads], axis=1)
        in_maps.append({
            "xT": np.ascontiguousarray(x[b].T),
            "wqkv": np.ascontiguousarray(np.concatenate([wq, wk, wv], axis=1)),
            "wp": np.ascontiguousarray(Wp[g * HG * DH:(g + 1) * HG * DH, :]),
            "tril": tril_h,
            "ones8": np.ones((128, HG), np.float32),
        })
    return in_maps


_LAST_RESULTS = {}


def kernel(x, Wq, Wk, Wv, Wp, bp):
    x = np.asarray(x, np.float32)
    Wq = np.asarray(Wq, np.float32)
    Wk = np.asarray(Wk, np.float32)
    Wv = np.asarray(Wv, np.float32)
    Wp = np.asarray(Wp, np.float32)
    bp = np.asarray(bp, np.float32)

    nc = _get_nc()
    in_maps = _make_in_maps(x, Wq, Wk, Wv, Wp)
    res = bass_utils.run_bass_kernel_spmd(
        nc, in_maps, core_ids=list(range(8)), trace=TRACE)
    _LAST_RESULTS["res"] = res

    out = np.empty((B, T, C), np.float32)
    for b in range(B):
        out[b] = res.results[2 * b]["out"] + res.results[2 * b + 1]["out"] + bp
    return out
